# revision 21
# baseline (speedup 1.0000x reference)
"""Masked cross-attention kernel for Trainium2 (8 NeuronCores).

Per batch b:  S = O @ E^T  (masked cols >= L_b) ; A = softmax(S) ; C = A @ E.
Outputs: (context [B,1024,256], attn [B,1024,4096]).

Distribution: batches are paired big+small by length; each pair is split
across two cores (4 o-chunks of the big batch + 4 o-chunks of the small
batch per core). All cores run ONE uniform program on widths (W1, W2) =
padded max big/small lengths, so per-core work is ~balanced while the
instruction stream is identical (SPMD requirement).

Per-core pipeline (slot = one 128-row o-chunk of one batch):
  - mm1 (fp16 x fp16, N<=512 tiles, PSUM quarters of <=1024 cols)
  - online softmax across quarters: DVE running max; ACT Exp with
    per-partition bias evacuates PSUM and emits row sums (accum_out);
    padded-column sum contribution subtracted in closed form (the pad
    columns are copies of column 0).
  - normalize in place (DVE tensor_scalar 2x) -> attn rows DMA out (fp32)
  - cast to bf16 (DVE/ACT halves)
  - A^T via REGULAR bf16 matmuls against an identity moving operand
    (pipelines + keeps the PE HAM-warm, unlike transpose-mode), skewed two
    slots behind mm1 so the PE never waits on the softmax chain.
  - mm2 (bf16): C^T += E_chunk-stationary @ A^T, per 4-slot segment;
    16 fp32 PE transposes fix C^T -> C.
Host prep rounds O,E to fp16 for mm1 (attn rel err ~1.2e-3) and bf16 for
mm2 (context rel err ~2.3e-3); E^T pad columns replicate column 0 so the
row max over the padded width equals the exact masked row max.
"""

import numpy as np
from contextlib import ExitStack

import concourse.bass as bass
import concourse.bacc as bacc
import concourse.tile as tile
from concourse import mybir
from concourse.masks import make_identity
from concourse.bass_utils import run_bass_kernel_spmd

import ml_dtypes

F32 = mybir.dt.float32
F16 = mybir.dt.float16
BF16 = mybir.dt.bfloat16
P = 128
N_CORES = 8
SKEW = 2          # transpose stage runs this many slots behind mm1/softmax

_program_cache = {}
_last_in_maps = None


def _tile_split(width: int) -> list[int]:
    tiles, rem = [], width
    while rem > 640:
        tiles.append(512)
        rem -= 512
    if rem == 640:
        tiles += [384, 256]
    else:
        tiles.append(rem)  # 256..640
    return tiles


def _quarters(width: int):
    qs, off, cur, cw = [], 0, [], 0
    for w in _tile_split(width):
        if cw + w > 1024:
            qs.append((off, cur))
            off += cw
            cur, cw = [], 0
        cur.append(w)
        cw += w
    qs.append((off, cur))
    return qs


def _build_program(W1: int, W2: int, OUT: int, H: int):
    assert H == 256 and OUT == 1024
    nKC = H // P
    NSEG_OC = 4                      # o-chunks per segment (per core)
    SEGW = NSEG_OC * P               # 512 output rows per segment

    nc = bacc.Bacc("TRN2", target_bir_lowering=False, debug=False,
                   num_devices=N_CORES)

    segs = []
    for name, W in (("A", W1), ("B", W2)):
        seg = {
            "name": name, "W": W, "nIC": W // P, "qs": _quarters(W),
            "oT_d": nc.dram_tensor(f"oT{name}", [H, SEGW], F16, kind="ExternalInput"),
            "eT_d": nc.dram_tensor(f"eT{name}", [H, W], F16, kind="ExternalInput"),
            "e_d": nc.dram_tensor(f"e{name}", [W, H], BF16, kind="ExternalInput"),
            "cnt_d": nc.dram_tensor(f"cnt{name}", [P, 1], F32, kind="ExternalInput"),
            "attn_d": nc.dram_tensor(f"attn{name}", [SEGW, W], F32, kind="ExternalOutput"),
            "ctx_d": nc.dram_tensor(f"ctx{name}", [SEGW, H], F32, kind="ExternalOutput"),
        }
        segs.append(seg)

    with ExitStack() as ctx:
        tc = ctx.enter_context(tile.TileContext(nc))
        consts = ctx.enter_context(tc.tile_pool(name="consts", bufs=1))
        big = ctx.enter_context(tc.tile_pool(name="big", bufs=1))
        upool = ctx.enter_context(tc.tile_pool(name="upool", bufs=3))
        bfpool = ctx.enter_context(tc.tile_pool(name="bfpool", bufs=SKEW + 1))
        stats = ctx.enter_context(tc.tile_pool(name="stats", bufs=2))
        csb = ctx.enter_context(tc.tile_pool(name="csb", bufs=2))
        sps = ctx.enter_context(tc.tile_pool(name="sps", bufs=2, space="PSUM"))
        atps = ctx.enter_context(tc.tile_pool(name="atps", bufs=2, space="PSUM"))
        ctps = ctx.enter_context(tc.tile_pool(name="ctps", bufs=2, space="PSUM"))

        ident_bf = consts.tile([P, P], BF16)
        make_identity(nc, ident_bf)
        for si, seg in enumerate(segs):
            nm, W, nIC = seg["name"], seg["W"], seg["nIC"]
            seg["oT_sb"] = big.tile([P, nKC, SEGW], F16, name=f"oT{nm}_sb", tag=f"oT{nm}")
            seg["e_sb"] = big.tile([P, nIC, H], BF16, name=f"e{nm}_sb", tag=f"e{nm}")
            seg["cnt_sb"] = consts.tile([P, 1], F32, name=f"cnt{nm}_sb", tag=f"cnt{nm}")
            seg["atT"] = [big.tile([P, nIC, P], BF16, name=f"atT{nm}{j}",
                                   tag=f"atT{nm}{j}") for j in range(NSEG_OC)]
            seg["ldq"] = nc.sync if si == 0 else nc.scalar
        # eT is loaded as per-(512-col-tile, k-chunk) pieces (~128KB each) in
        # strict need order across both HWDGE queues, so the mm1 stream starts
        # ~4us earlier and chases the DMA stream piece by piece.
        for seg in segs:
            nm = seg["name"]
            seg["eT_full"] = seg["eT_d"].ap().rearrange("(k p) i -> p k i", p=P)
            seg["eT_t"] = []
            for q, (qoff, qtiles) in enumerate(seg["qs"]):
                row = []
                toff = 0
                for ti, w in enumerate(qtiles):
                    ks = [big.tile([P, w], F16, name=f"eT{nm}q{q}t{ti}k{k}",
                                   tag=f"eT{nm}q{q}t{ti}k{k}")
                          for k in range(nKC)]
                    row.append((ks, qoff + toff, w))
                    toff += w
                seg["eT_t"].append(row)
        sA, sB = segs[0], segs[1]

        def ld_pieces(seg, q):
            for (ks, off, w) in seg["eT_t"][q]:
                for k in range(nKC):
                    eng = nc.sync if k == 0 else nc.scalar
                    eng.dma_start(out=ks[k],
                                  in_=seg["eT_full"][:, k, off:off + w])

        nc.scalar.dma_start(out=sA["oT_sb"],
                            in_=sA["oT_d"].ap().rearrange("(k p) o -> p k o", p=P))
        ld_pieces(sA, 0)
        nc.sync.dma_start(out=sB["oT_sb"],
                          in_=sB["oT_d"].ap().rearrange("(k p) o -> p k o", p=P))
        nc.scalar.dma_start(out=sA["cnt_sb"], in_=sA["cnt_d"][:])
        nc.scalar.dma_start(out=sB["cnt_sb"], in_=sB["cnt_d"][:])
        for q in range(1, len(sA["eT_t"])):
            ld_pieces(sA, q)
        for q in range(len(sB["eT_t"])):
            ld_pieces(sB, q)
        nc.sync.dma_start(out=sA["e_sb"],
                          in_=sA["e_d"].ap().rearrange("(c p) h -> p c h", p=P))
        nc.scalar.dma_start(out=sB["e_sb"],
                            in_=sB["e_d"].ap().rearrange("(c p) h -> p c h", p=P))
        for seg in segs:
            seg["eT_t"] = [[ks for (ks, _, _) in row] for row in seg["eT_t"]]


        NSLOT = 2 * NSEG_OC
        # A0 A1 B0 A2 B1 A3 B2 B3: A's first slots run while B's loads land
        order = [(0, 0), (0, 1), (1, 0), (0, 2), (1, 1), (0, 3), (1, 2), (1, 3)]
        slot_seg = [segs[si] for si, _ in order]
        slot_j = [j for _, j in order]
        slot_u = [None] * NSLOT
        slot_rcp = [None] * NSLOT
        slot_abf = [None] * NSLOT

        def mm1_softmax(s):
            seg = slot_seg[s]
            W, qs = seg["W"], seg["qs"]
            nQ = len(qs)
            lcol = slot_j[s] * P
            u_sb = upool.tile([P, W], F32, name=f"u_{s}", tag="u")
            nm = stats.tile([P, 1], F32, name=f"nm{s}", tag="nm")
            su = stats.tile([P, 4], F32, name=f"su{s}", tag="su")
            # Single softmax reference: the max of quarter 0 (>= 1024 Gaussian
            # scores, so later quarters cannot exceed it by anywhere near
            # exp's fp32 range; softmax is invariant to the reference).
            for q, (qoff, qtiles) in enumerate(qs):
                qw = sum(qtiles)
                s_ps = sps.tile([P, 1024], F32, name=f"s{s}_{q}", tag="s")
                for k in range(nKC):
                    toff = 0
                    for ti, w in enumerate(qtiles):
                        nc.tensor.matmul(
                            s_ps[:, toff:toff + w],
                            seg["oT_sb"][:, k, lcol:lcol + P],
                            seg["eT_t"][q][ti][k],
                            start=(k == 0), stop=(k == nKC - 1))
                        toff += w
                if q == 0:
                    # reference = max over the first 512 scores (still >= 512
                    # Gaussian samples -> later values can't exceed it by
                    # anywhere near exp's fp32 range; softmax is shift-invariant)
                    nc.vector.reduce_max(nm, s_ps[:, :min(qw, 512)],
                                         axis=mybir.AxisListType.X, negate=True)
                nc.scalar.activation(out=u_sb[:, qoff:qoff + qw], in_=s_ps[:, :qw],
                                     func=mybir.ActivationFunctionType.Exp,
                                     bias=nm, scale=1.0,
                                     accum_out=su[:, q:q + 1])
            ssum = stats.tile([P, 1], F32, name=f"ssum{s}", tag="ssum")
            nc.vector.reduce_sum(ssum, su[:, :nQ], axis=mybir.AxisListType.X)
            corr = stats.tile([P, 1], F32, name=f"corr{s}", tag="corr")
            nc.vector.tensor_tensor(corr, u_sb[:, 0:1], seg["cnt_sb"],
                                    mybir.AluOpType.mult)
            sv = stats.tile([P, 1], F32, name=f"sv{s}", tag="sv")
            nc.vector.tensor_tensor(sv, ssum, corr, mybir.AluOpType.subtract)
            rcp = stats.tile([P, 1], F32, name=f"rcp{s}", tag="rcp")
            nc.vector.reciprocal(rcp, sv)
            slot_u[s] = u_sb
            slot_rcp[s] = rcp

        def finish(s):
            seg = slot_seg[s]
            W = seg["W"]
            lcol = slot_j[s] * P
            u_sb, rcp = slot_u[s], slot_rcp[s]
            nc.vector.tensor_scalar_mul(u_sb, u_sb, rcp)
            half = (W // 2 // P) * P
            nc.sync.dma_start(out=seg["attn_d"][lcol:lcol + P, :half],
                              in_=u_sb[:, :half])
            nc.scalar.dma_start(out=seg["attn_d"][lcol:lcol + P, half:],
                                in_=u_sb[:, half:])
            hs = min(((W // 2 + 511) // 512) * 512, W)
            if seg["name"] == "B" or hs == W:
                a_bf = bfpool.tile([P, W], BF16, name=f"abf{s}", tag="abf")
                nc.gpsimd.tensor_copy(out=a_bf, in_=u_sb)
                slot_abf[s] = [(0, W, a_bf)]
            else:
                a1 = bfpool.tile([P, hs], BF16, name=f"abf{s}a", tag="abfa")
                a2 = bfpool.tile([P, W - hs], BF16, name=f"abf{s}b", tag="abfb")
                nc.vector.tensor_copy(out=a1, in_=u_sb[:, :hs])
                nc.scalar.copy(out=a2, in_=u_sb[:, hs:])
                slot_abf[s] = [(0, hs, a1), (hs, W - hs, a2)]

        def transposes(s):
            seg = slot_seg[s]
            lcol = slot_j[s] * P
            gidx = 0
            for (hoff, hww, a_bf) in slot_abf[s]:
                ic0 = hoff // P
                nh_ic = hww // P
                for g in range(0, nh_ic, 4):
                    ng = min(4, nh_ic - g)
                    at_ps = atps.tile([P, 512], F32, name=f"at{s}_{ic0+g}", tag="at")
                    for j in range(ng):
                        nc.tensor.matmul(at_ps[:, j * P:(j + 1) * P],
                                         a_bf[:, (g + j) * P:(g + j + 1) * P],
                                         ident_bf)
                    dest = seg["atT"][slot_j[s]][:, ic0 + g:ic0 + g + ng, :]
                    srcv = at_ps[:, :ng * P].rearrange("p (a b) -> p a b", a=ng)
                    if gidx % 2 == 0:
                        nc.vector.tensor_copy(out=dest, in_=srcv)
                    else:
                        nc.scalar.copy(out=dest, in_=srcv)
                    gidx += 1

        def mm2_slot(s):
            seg = slot_seg[s]
            j = slot_j[s]
            nIC = seg["nIC"]
            atT = seg["atT"][j]
            ct_ps = ctps.tile([P, H], F32, name=f"ct{s}", tag="ct")
            for ic in range(nIC):
                nc.tensor.matmul(ct_ps, atT[:, ic, :], seg["e_sb"][:, ic, :],
                                 start=(ic == 0), stop=(ic == nIC - 1))
            c_sb = csb.tile([P, H], F32, name=f"csb{s}", tag="csb")
            if s % 2 == 0:
                nc.vector.tensor_copy(out=c_sb, in_=ct_ps)
            else:
                nc.scalar.copy(out=c_sb, in_=ct_ps)
            cq = nc.sync if s % 2 == 0 else nc.scalar
            cq.dma_start(out=seg["ctx_d"][j * P:(j + 1) * P, :], in_=c_sb)

        for t in range(NSLOT + SKEW):
            if t < NSLOT:
                mm1_softmax(t)
            if 0 <= t - 1 < NSLOT:
                finish(t - 1)
            if t >= SKEW:
                s = t - SKEW
                transposes(s)
                mm2_slot(s)

    nc.compile()
    return nc


def _ceil128(x):
    return max(256, ((x + P - 1) // P) * P)


def kernel(output: np.ndarray, encoder_outputs: np.ndarray,
           lengths: np.ndarray) -> tuple[np.ndarray, np.ndarray]:
    global _last_in_maps
    B, OUT, H = output.shape
    IN = encoder_outputs.shape[1]
    assert B == N_CORES
    lens = [max(1, min(int(l), IN)) for l in np.asarray(lengths)]

    order = sorted(range(B), key=lambda b: -lens[b])
    bigs, smalls = order[:4], order[7:3:-1]      # pair big[i] with small[i]
    W1 = _ceil128(max(lens[b] for b in bigs))
    W2 = _ceil128(max(lens[b] for b in smalls))

    key = (W1, W2, OUT, H)
    if key not in _program_cache:
        _program_cache[key] = _build_program(W1, W2, OUT, H)
    nc = _program_cache[key]

    def seg_inputs(b, W, nm):
        L = lens[b]
        E = np.asarray(encoder_outputs[b], dtype=np.float32)
        eT = np.empty((H, W), np.float16)
        eT[:, :L] = E[:L].T
        eT[:, L:] = E[0:1].T
        e_bf = np.zeros((W, H), ml_dtypes.bfloat16)
        e_bf[:L] = E[:L].astype(ml_dtypes.bfloat16)
        return {
            f"eT{nm}": eT, f"e{nm}": e_bf,
            f"cnt{nm}": np.full((P, 1), float(W - L), np.float32),
        }

    in_maps = []
    placement = []   # per core: (bigbatch, row slice, smallbatch, row slice)
    for p in range(4):
        bA, bB = bigs[p], smalls[p]
        iA = seg_inputs(bA, W1, "A")
        iB = seg_inputs(bB, W2, "B")
        OA = np.asarray(output[bA], np.float32).T.astype(np.float16)  # [H, 1024]
        OB = np.asarray(output[bB], np.float32).T.astype(np.float16)
        for half in range(2):
            sl = slice(half * 512, (half + 1) * 512)
            m = {"oTA": np.ascontiguousarray(OA[:, sl]),
                 "oTB": np.ascontiguousarray(OB[:, sl])}
            m.update(iA)
            m.update(iB)
            in_maps.append(m)
            placement.append((bA, sl, bB, sl))

    _last_in_maps = in_maps
    res = run_bass_kernel_spmd(nc, in_maps, list(range(N_CORES)))

    attn = np.zeros((B, OUT, IN), np.float32)
    context = np.empty((B, OUT, H), np.float32)
    for c, (bA, slA, bB, slB) in enumerate(placement):
        r = res.results[c]
        LA, LB = lens[bA], lens[bB]
        attn[bA, slA, :LA] = r["attnA"][:, :LA]
        attn[bB, slB, :LB] = r["attnB"][:, :LB]
        context[bA, slA] = r["ctxA"]
        context[bB, slB] = r["ctxB"]
    return (context, attn)


# revision 23
# speedup vs baseline: 1.0248x; 1.0248x over previous
"""Masked cross-attention kernel for Trainium2 (8 NeuronCores).

Per batch b:  S = O @ E^T  (masked cols >= L_b) ; A = softmax(S) ; C = A @ E.
Outputs: (context [B,1024,256], attn [B,1024,4096]).

Distribution: batches are paired big+small by length; each pair is split
across two cores (4 o-chunks of the big batch + 4 o-chunks of the small
batch per core). All cores run ONE uniform program on widths (W1, W2) =
padded max big/small lengths, so per-core work is ~balanced while the
instruction stream is identical (SPMD requirement).

Per-core pipeline (slot = one 128-row o-chunk of one batch):
  - mm1 (fp16 x fp16, N<=512 tiles, PSUM quarters of <=1024 cols)
  - online softmax across quarters: DVE running max; ACT Exp with
    per-partition bias evacuates PSUM and emits row sums (accum_out);
    padded-column sum contribution subtracted in closed form (the pad
    columns are copies of column 0).
  - normalize in place (DVE tensor_scalar 2x) -> attn rows DMA out (fp32)
  - cast to bf16 (DVE/ACT halves)
  - A^T via REGULAR bf16 matmuls against an identity moving operand
    (pipelines + keeps the PE HAM-warm, unlike transpose-mode), skewed two
    slots behind mm1 so the PE never waits on the softmax chain.
  - mm2 (bf16): C^T += E_chunk-stationary @ A^T, per 4-slot segment;
    16 fp32 PE transposes fix C^T -> C.
Host prep rounds O,E to fp16 for mm1 (attn rel err ~1.2e-3) and bf16 for
mm2 (context rel err ~2.3e-3); E^T pad columns replicate column 0 so the
row max over the padded width equals the exact masked row max.
"""

import numpy as np
from contextlib import ExitStack

import concourse.bass as bass
import concourse.bacc as bacc
import concourse.tile as tile
from concourse import mybir
from concourse.masks import make_identity
from concourse.bass_utils import run_bass_kernel_spmd

import ml_dtypes

F32 = mybir.dt.float32
F16 = mybir.dt.float16
BF16 = mybir.dt.bfloat16
P = 128
N_CORES = 8
SKEW = 2          # transpose stage runs this many slots behind mm1/softmax

_program_cache = {}
_last_in_maps = None


def _tile_split(width: int) -> list[int]:
    tiles, rem = [], width
    while rem > 640:
        tiles.append(512)
        rem -= 512
    if rem == 640:
        tiles += [384, 256]
    else:
        tiles.append(rem)  # 256..640
    return tiles


def _quarters(width: int):
    qs, off, cur, cw = [], 0, [], 0
    for w in _tile_split(width):
        if cw + w > 1024:
            qs.append((off, cur))
            off += cw
            cur, cw = [], 0
        cur.append(w)
        cw += w
    qs.append((off, cur))
    return qs


def _build_program(W1: int, W2: int, OUT: int, H: int):
    assert H == 256 and OUT == 1024
    nKC = H // P
    NSEG_OC = 4                      # o-chunks per segment (per core)
    SEGW = NSEG_OC * P               # 512 output rows per segment

    nc = bacc.Bacc("TRN2", target_bir_lowering=False, debug=False,
                   num_devices=N_CORES)

    segs = []
    for name, W in (("A", W1), ("B", W2)):
        seg = {
            "name": name, "W": W, "nIC": W // P, "qs": _quarters(W),
            "oT_d": nc.dram_tensor(f"oT{name}", [H, SEGW], F16, kind="ExternalInput"),
            "eT_d": nc.dram_tensor(f"eT{name}", [H, W], F16, kind="ExternalInput"),
            "e_d": nc.dram_tensor(f"e{name}", [W, H], BF16, kind="ExternalInput"),
            "cnt_d": nc.dram_tensor(f"cnt{name}", [P, 1], F32, kind="ExternalInput"),
            "attn_d": nc.dram_tensor(f"attn{name}", [SEGW, W], F32, kind="ExternalOutput"),
            "ctx_d": nc.dram_tensor(f"ctx{name}", [SEGW, H], F32, kind="ExternalOutput"),
        }
        segs.append(seg)

    with ExitStack() as ctx:
        tc = ctx.enter_context(tile.TileContext(nc))
        consts = ctx.enter_context(tc.tile_pool(name="consts", bufs=1))
        big = ctx.enter_context(tc.tile_pool(name="big", bufs=1))
        upool = ctx.enter_context(tc.tile_pool(name="upool", bufs=3))
        bfpool = ctx.enter_context(tc.tile_pool(name="bfpool", bufs=SKEW + 1))
        stats = ctx.enter_context(tc.tile_pool(name="stats", bufs=2))
        csb = ctx.enter_context(tc.tile_pool(name="csb", bufs=2))
        sps = ctx.enter_context(tc.tile_pool(name="sps", bufs=3, space="PSUM"))
        atps = ctx.enter_context(tc.tile_pool(name="atps", bufs=2, space="PSUM"))

        ident_bf = consts.tile([P, P], BF16)
        make_identity(nc, ident_bf)
        for si, seg in enumerate(segs):
            nm, W, nIC = seg["name"], seg["W"], seg["nIC"]
            seg["oT_sb"] = big.tile([P, nKC, SEGW], F16, name=f"oT{nm}_sb", tag=f"oT{nm}")
            seg["e_sb"] = big.tile([P, nIC, H], BF16, name=f"e{nm}_sb", tag=f"e{nm}")
            seg["cnt_sb"] = consts.tile([P, 1], F32, name=f"cnt{nm}_sb", tag=f"cnt{nm}")
            seg["atT"] = [big.tile([P, nIC, P], BF16, name=f"atT{nm}{j}",
                                   tag=f"atT{nm}{j}") for j in range(NSEG_OC)]
            seg["ldq"] = nc.sync if si == 0 else nc.scalar
        # eT tiles are per (quarter, k-chunk) so k0 matmuls can start while the
        # k1 half is still in flight; loads are issued in need order, spread
        # across both HWDGE queues.
        for seg in segs:
            nm = seg["name"]
            seg["eT_full"] = seg["eT_d"].ap().rearrange("(k p) i -> p k i", p=P)
            seg["eT_q"] = []
            for q, (qoff, qtiles) in enumerate(seg["qs"]):
                qw = sum(qtiles)
                ks = [big.tile([P, qw], F16, name=f"eT{nm}q{q}k{k}",
                               tag=f"eT{nm}q{q}k{k}") for k in range(nKC)]
                seg["eT_q"].append((ks, qoff, qw))
        sA, sB = segs[0], segs[1]

        def ld_eT(seg, q, k, eng):
            ks, qoff, qw = seg["eT_q"][q]
            eng.dma_start(out=ks[k],
                          in_=seg["eT_full"][:, k, qoff:qoff + qw])

        nc.scalar.dma_start(out=sA["oT_sb"],
                            in_=sA["oT_d"].ap().rearrange("(k p) o -> p k o", p=P))
        ld_eT(sA, 0, 0, nc.sync)
        ld_eT(sA, 0, 1, nc.scalar)
        nc.sync.dma_start(out=sB["oT_sb"],
                          in_=sB["oT_d"].ap().rearrange("(k p) o -> p k o", p=P))
        if len(sA["eT_q"]) > 1:
            ld_eT(sA, 1, 0, nc.sync)
            ld_eT(sA, 1, 1, nc.scalar)
        ld_eT(sB, 0, 0, nc.sync)
        ld_eT(sB, 0, 1, nc.scalar)
        nc.sync.dma_start(out=sA["cnt_sb"], in_=sA["cnt_d"][:])
        nc.sync.dma_start(out=sB["cnt_sb"], in_=sB["cnt_d"][:])
        for q in range(2, len(sA["eT_q"])):
            ld_eT(sA, q, 0, nc.sync)
            ld_eT(sA, q, 1, nc.scalar)
        for q in range(1, len(sB["eT_q"])):
            ld_eT(sB, q, 0, nc.sync)
            ld_eT(sB, q, 1, nc.scalar)
        nc.sync.dma_start(out=sA["e_sb"],
                          in_=sA["e_d"].ap().rearrange("(c p) h -> p c h", p=P))
        nc.scalar.dma_start(out=sB["e_sb"],
                            in_=sB["e_d"].ap().rearrange("(c p) h -> p c h", p=P))
        for seg in segs:
            seg["eT_q"] = [ks for (ks, _, _) in seg["eT_q"]]


        NSLOT = 2 * NSEG_OC
        # A0 A1 B0 A2 B1 A3 B2 B3: A's first slots run while B's loads land
        order = [(0, 0), (0, 1), (1, 0), (0, 2), (1, 1), (0, 3), (1, 2), (1, 3)]
        slot_seg = [segs[si] for si, _ in order]
        slot_j = [j for _, j in order]
        slot_u = [None] * NSLOT
        slot_rcp = [None] * NSLOT
        slot_abf = [None] * NSLOT

        def mm1_softmax(s):
            seg = slot_seg[s]
            W, qs = seg["W"], seg["qs"]
            nQ = len(qs)
            lcol = slot_j[s] * P
            u_sb = upool.tile([P, W], F32, name=f"u_{s}", tag="u")
            nm = stats.tile([P, 1], F32, name=f"nm{s}", tag="nm")
            su = stats.tile([P, 4], F32, name=f"su{s}", tag="su")
            # Single softmax reference: the max of quarter 0 (>= 1024 Gaussian
            # scores, so later quarters cannot exceed it by anywhere near
            # exp's fp32 range; softmax is invariant to the reference).
            for q, (qoff, qtiles) in enumerate(qs):
                qw = sum(qtiles)
                s_ps = sps.tile([P, 1024], F32, name=f"s{s}_{q}", tag="s")
                for k in range(nKC):
                    toff = 0
                    for w in qtiles:
                        nc.tensor.matmul(
                            s_ps[:, toff:toff + w],
                            seg["oT_sb"][:, k, lcol:lcol + P],
                            seg["eT_q"][q][k][:, toff:toff + w],
                            start=(k == 0), stop=(k == nKC - 1))
                        toff += w
                if q == 0:
                    # reference = max over the first 512 scores (still >= 512
                    # Gaussian samples -> later values can't exceed it by
                    # anywhere near exp's fp32 range; softmax is shift-invariant)
                    nc.vector.reduce_max(nm, s_ps[:, :min(qw, 512)],
                                         axis=mybir.AxisListType.X, negate=True)
                nc.scalar.activation(out=u_sb[:, qoff:qoff + qw], in_=s_ps[:, :qw],
                                     func=mybir.ActivationFunctionType.Exp,
                                     bias=nm, scale=1.0,
                                     accum_out=su[:, q:q + 1])
            ssum = stats.tile([P, 1], F32, name=f"ssum{s}", tag="ssum")
            nc.vector.reduce_sum(ssum, su[:, :nQ], axis=mybir.AxisListType.X)
            corr = stats.tile([P, 1], F32, name=f"corr{s}", tag="corr")
            nc.vector.tensor_tensor(corr, u_sb[:, 0:1], seg["cnt_sb"],
                                    mybir.AluOpType.mult)
            sv = stats.tile([P, 1], F32, name=f"sv{s}", tag="sv")
            nc.vector.tensor_tensor(sv, ssum, corr, mybir.AluOpType.subtract)
            rcp = stats.tile([P, 1], F32, name=f"rcp{s}", tag="rcp")
            nc.vector.reciprocal(rcp, sv)
            slot_u[s] = u_sb
            slot_rcp[s] = rcp

        def finish(s):
            seg = slot_seg[s]
            W = seg["W"]
            lcol = slot_j[s] * P
            u_sb, rcp = slot_u[s], slot_rcp[s]
            nc.vector.tensor_scalar_mul(u_sb, u_sb, rcp)
            dq = nc.sync if s % 2 == 0 else nc.scalar
            dq.dma_start(out=seg["attn_d"][lcol:lcol + P, :], in_=u_sb)
            hs = min(((W // 2 + 511) // 512) * 512, W)
            if seg["name"] == "B" or hs == W:
                a_bf = bfpool.tile([P, W], BF16, name=f"abf{s}", tag="abf")
                nc.gpsimd.tensor_copy(out=a_bf, in_=u_sb)
                slot_abf[s] = [(0, W, a_bf)]
            else:
                a1 = bfpool.tile([P, hs], BF16, name=f"abf{s}a", tag="abfa")
                a2 = bfpool.tile([P, W - hs], BF16, name=f"abf{s}b", tag="abfb")
                nc.vector.tensor_copy(out=a1, in_=u_sb[:, :hs])
                nc.scalar.copy(out=a2, in_=u_sb[:, hs:])
                slot_abf[s] = [(0, hs, a1), (hs, W - hs, a2)]

        def transposes(s):
            seg = slot_seg[s]
            lcol = slot_j[s] * P
            gidx = 0
            for (hoff, hww, a_bf) in slot_abf[s]:
                ic0 = hoff // P
                nh_ic = hww // P
                for g in range(0, nh_ic, 4):
                    ng = min(4, nh_ic - g)
                    at_ps = atps.tile([P, 512], F32, name=f"at{s}_{ic0+g}", tag="at")
                    for j in range(ng):
                        nc.tensor.matmul(at_ps[:, j * P:(j + 1) * P],
                                         a_bf[:, (g + j) * P:(g + j + 1) * P],
                                         ident_bf)
                    dest = seg["atT"][slot_j[s]][:, ic0 + g:ic0 + g + ng, :]
                    srcv = at_ps[:, :ng * P].rearrange("p (a b) -> p a b", a=ng)
                    if gidx % 2 == 0:
                        nc.vector.tensor_copy(out=dest, in_=srcv)
                    else:
                        nc.scalar.copy(out=dest, in_=srcv)
                    gidx += 1

        def mm2_slot(s):
            seg = slot_seg[s]
            j = slot_j[s]
            nIC = seg["nIC"]
            atT = seg["atT"][j]
            ct_ps = atps.tile([P, H], F32, name=f"ct{s}", tag="at")
            for ic in range(nIC):
                nc.tensor.matmul(ct_ps, atT[:, ic, :], seg["e_sb"][:, ic, :],
                                 start=(ic == 0), stop=(ic == nIC - 1))
            c_sb = csb.tile([P, H], F32, name=f"csb{s}", tag="csb")
            if s % 2 == 0:
                nc.vector.tensor_copy(out=c_sb, in_=ct_ps)
            else:
                nc.scalar.copy(out=c_sb, in_=ct_ps)
            cq = nc.sync if s % 2 == 0 else nc.scalar
            cq.dma_start(out=seg["ctx_d"][j * P:(j + 1) * P, :], in_=c_sb)

        for t in range(NSLOT + SKEW):
            if t < NSLOT:
                mm1_softmax(t)
            if 0 <= t - 1 < NSLOT:
                finish(t - 1)
            if t >= SKEW:
                s = t - SKEW
                transposes(s)
                mm2_slot(s)

    nc.compile()
    return nc


def _ceil128(x):
    return max(256, ((x + P - 1) // P) * P)


def kernel(output: np.ndarray, encoder_outputs: np.ndarray,
           lengths: np.ndarray) -> tuple[np.ndarray, np.ndarray]:
    global _last_in_maps
    B, OUT, H = output.shape
    IN = encoder_outputs.shape[1]
    assert B == N_CORES
    lens = [max(1, min(int(l), IN)) for l in np.asarray(lengths)]

    order = sorted(range(B), key=lambda b: -lens[b])
    bigs, smalls = order[:4], order[7:3:-1]      # pair big[i] with small[i]
    W1 = _ceil128(max(lens[b] for b in bigs))
    W2 = _ceil128(max(lens[b] for b in smalls))

    key = (W1, W2, OUT, H)
    if key not in _program_cache:
        _program_cache[key] = _build_program(W1, W2, OUT, H)
    nc = _program_cache[key]

    def seg_inputs(b, W, nm):
        L = lens[b]
        E = np.asarray(encoder_outputs[b], dtype=np.float32)
        eT = np.empty((H, W), np.float16)
        eT[:, :L] = E[:L].T
        eT[:, L:] = E[0:1].T
        e_bf = np.zeros((W, H), ml_dtypes.bfloat16)
        e_bf[:L] = E[:L].astype(ml_dtypes.bfloat16)
        return {
            f"eT{nm}": eT, f"e{nm}": e_bf,
            f"cnt{nm}": np.full((P, 1), float(W - L), np.float32),
        }

    in_maps = []
    placement = []   # per core: (bigbatch, row slice, smallbatch, row slice)
    for p in range(4):
        bA, bB = bigs[p], smalls[p]
        iA = seg_inputs(bA, W1, "A")
        iB = seg_inputs(bB, W2, "B")
        OA = np.asarray(output[bA], np.float32).T.astype(np.float16)  # [H, 1024]
        OB = np.asarray(output[bB], np.float32).T.astype(np.float16)
        for half in range(2):
            sl = slice(half * 512, (half + 1) * 512)
            m = {"oTA": np.ascontiguousarray(OA[:, sl]),
                 "oTB": np.ascontiguousarray(OB[:, sl])}
            m.update(iA)
            m.update(iB)
            in_maps.append(m)
            placement.append((bA, sl, bB, sl))

    _last_in_maps = in_maps
    res = run_bass_kernel_spmd(nc, in_maps, list(range(N_CORES)))

    attn = np.zeros((B, OUT, IN), np.float32)
    context = np.empty((B, OUT, H), np.float32)
    for c, (bA, slA, bB, slB) in enumerate(placement):
        r = res.results[c]
        LA, LB = lens[bA], lens[bB]
        attn[bA, slA, :LA] = r["attnA"][:, :LA]
        attn[bB, slB, :LB] = r["attnB"][:, :LB]
        context[bA, slA] = r["ctxA"]
        context[bB, slB] = r["ctxB"]
    return (context, attn)


# revision 24
# speedup vs baseline: 1.1241x; 1.0969x over previous
"""Masked cross-attention kernel for Trainium2 (8 NeuronCores).

Per batch b:  S = O @ E^T  (masked cols >= L_b) ; A = softmax(S) ; C = A @ E.
Outputs: (context [B,1024,256], attn [B,1024,4096]).

Distribution: batches are paired big+small by length; each pair is split
across two cores (4 o-chunks of the big batch + 4 o-chunks of the small
batch per core). All cores run ONE uniform program on widths (W1, W2) =
padded max big/small lengths, so per-core work is ~balanced while the
instruction stream is identical (SPMD requirement).

Per-core pipeline (slot = one 128-row o-chunk of one batch):
  - mm1 (fp16 x fp16, N<=512 tiles, PSUM quarters of <=1024 cols)
  - online softmax across quarters: DVE running max; ACT Exp with
    per-partition bias evacuates PSUM and emits row sums (accum_out);
    padded-column sum contribution subtracted in closed form (the pad
    columns are copies of column 0).
  - normalize in place (DVE tensor_scalar 2x) -> attn rows DMA out (fp32)
  - cast to bf16 (DVE/ACT halves)
  - A^T via REGULAR bf16 matmuls against an identity moving operand
    (pipelines + keeps the PE HAM-warm, unlike transpose-mode), skewed two
    slots behind mm1 so the PE never waits on the softmax chain.
  - mm2 (bf16): C^T += E_chunk-stationary @ A^T, per 4-slot segment;
    16 fp32 PE transposes fix C^T -> C.
Host prep rounds O,E to fp16 for mm1 (attn rel err ~1.2e-3) and bf16 for
mm2 (context rel err ~2.3e-3); E^T pad columns replicate column 0 so the
row max over the padded width equals the exact masked row max.
"""

import numpy as np
from contextlib import ExitStack

import concourse.bass as bass
import concourse.bacc as bacc
import concourse.tile as tile
from concourse import mybir
from concourse.masks import make_identity
from concourse.bass_utils import run_bass_kernel_spmd

import ml_dtypes

F32 = mybir.dt.float32
F16 = mybir.dt.float16
BF16 = mybir.dt.bfloat16
P = 128
N_CORES = 8
SKEW = 2          # transpose stage runs this many slots behind mm1/softmax

_program_cache = {}
_last_in_maps = None


def _tile_split(width: int) -> list[int]:
    tiles, rem = [], width
    while rem > 640:
        tiles.append(512)
        rem -= 512
    if rem == 640:
        tiles += [384, 256]
    else:
        tiles.append(rem)  # 256..640
    return tiles


def _quarters(width: int):
    qs, off, cur, cw = [], 0, [], 0
    for w in _tile_split(width):
        if cw + w > 1024:
            qs.append((off, cur))
            off += cw
            cur, cw = [], 0
        cur.append(w)
        cw += w
    qs.append((off, cur))
    return qs


def _build_program(W1: int, W2: int, OUT: int, H: int):
    assert H == 256 and OUT == 1024
    nKC = H // P
    NSEG_OC = 4                      # o-chunks per segment (per core)
    SEGW = NSEG_OC * P               # 512 output rows per segment

    nc = bacc.Bacc("TRN2", target_bir_lowering=False, debug=False,
                   num_devices=N_CORES)

    segs = []
    for name, W in (("A", W1), ("B", W2)):
        seg = {
            "name": name, "W": W, "nIC": W // P, "qs": _quarters(W),
            "oT_d": nc.dram_tensor(f"oT{name}", [H, SEGW], F16, kind="ExternalInput"),
            "eT_d": nc.dram_tensor(f"eT{name}", [H, W], F16, kind="ExternalInput"),
            "e_d": nc.dram_tensor(f"e{name}", [W, H], BF16, kind="ExternalInput"),
            "cnt_d": nc.dram_tensor(f"cnt{name}", [P, 1], F32, kind="ExternalInput"),
            "attn_d": nc.dram_tensor(f"attn{name}", [SEGW, W], F32, kind="ExternalOutput"),
            "ctx_d": nc.dram_tensor(f"ctx{name}", [SEGW, H], F32, kind="ExternalOutput"),
        }
        segs.append(seg)

    with ExitStack() as ctx:
        tc = ctx.enter_context(tile.TileContext(nc))
        consts = ctx.enter_context(tc.tile_pool(name="consts", bufs=1))
        big = ctx.enter_context(tc.tile_pool(name="big", bufs=1))
        upool = ctx.enter_context(tc.tile_pool(name="upool", bufs=3))
        bfpool = ctx.enter_context(tc.tile_pool(name="bfpool", bufs=SKEW + 1))
        stats = ctx.enter_context(tc.tile_pool(name="stats", bufs=2))
        csb = ctx.enter_context(tc.tile_pool(name="csb", bufs=2))
        sps = ctx.enter_context(tc.tile_pool(name="sps", bufs=2, space="PSUM"))
        atps = ctx.enter_context(tc.tile_pool(name="atps", bufs=2, space="PSUM"))
        ctps = ctx.enter_context(tc.tile_pool(name="ctps", bufs=2, space="PSUM"))

        ident_bf = consts.tile([P, P], BF16)
        make_identity(nc, ident_bf)
        for si, seg in enumerate(segs):
            nm, W, nIC = seg["name"], seg["W"], seg["nIC"]
            seg["oT_sb"] = big.tile([P, nKC, SEGW], F16, name=f"oT{nm}_sb", tag=f"oT{nm}")
            seg["e_sb"] = big.tile([P, nIC, H], BF16, name=f"e{nm}_sb", tag=f"e{nm}")
            seg["cnt_sb"] = consts.tile([P, 1], F32, name=f"cnt{nm}_sb", tag=f"cnt{nm}")
            seg["atT"] = [big.tile([P, nIC, P], BF16, name=f"atT{nm}{j}",
                                   tag=f"atT{nm}{j}") for j in range(NSEG_OC)]
            seg["ldq"] = nc.sync if si == 0 else nc.scalar
        # eT tiles are per (quarter, k-chunk) so k0 matmuls can start while the
        # k1 half is still in flight; loads are issued in need order, spread
        # across both HWDGE queues.
        for seg in segs:
            nm = seg["name"]
            seg["eT_full"] = seg["eT_d"].ap().rearrange("(k p) i -> p k i", p=P)
            seg["eT_q"] = []
            for q, (qoff, qtiles) in enumerate(seg["qs"]):
                qw = sum(qtiles)
                ks = [big.tile([P, qw], F16, name=f"eT{nm}q{q}k{k}",
                               tag=f"eT{nm}q{q}k{k}") for k in range(nKC)]
                seg["eT_q"].append((ks, qoff, qw))
        sA, sB = segs[0], segs[1]

        def ld_eT(seg, q, k, eng):
            ks, qoff, qw = seg["eT_q"][q]
            eng.dma_start(out=ks[k],
                          in_=seg["eT_full"][:, k, qoff:qoff + qw])

        nc.scalar.dma_start(out=sA["oT_sb"],
                            in_=sA["oT_d"].ap().rearrange("(k p) o -> p k o", p=P))
        ld_eT(sA, 0, 0, nc.sync)
        ld_eT(sA, 0, 1, nc.scalar)
        nc.sync.dma_start(out=sB["oT_sb"],
                          in_=sB["oT_d"].ap().rearrange("(k p) o -> p k o", p=P))
        if len(sA["eT_q"]) > 1:
            ld_eT(sA, 1, 0, nc.sync)
            ld_eT(sA, 1, 1, nc.scalar)
        ld_eT(sB, 0, 0, nc.sync)
        ld_eT(sB, 0, 1, nc.scalar)
        nc.sync.dma_start(out=sA["cnt_sb"], in_=sA["cnt_d"][:])
        nc.sync.dma_start(out=sB["cnt_sb"], in_=sB["cnt_d"][:])
        for q in range(2, len(sA["eT_q"])):
            ld_eT(sA, q, 0, nc.sync)
            ld_eT(sA, q, 1, nc.scalar)
        for q in range(1, len(sB["eT_q"])):
            ld_eT(sB, q, 0, nc.sync)
            ld_eT(sB, q, 1, nc.scalar)
        nc.sync.dma_start(out=sA["e_sb"],
                          in_=sA["e_d"].ap().rearrange("(c p) h -> p c h", p=P))
        nc.scalar.dma_start(out=sB["e_sb"],
                            in_=sB["e_d"].ap().rearrange("(c p) h -> p c h", p=P))
        for seg in segs:
            seg["eT_q"] = [ks for (ks, _, _) in seg["eT_q"]]


        NSLOT = 2 * NSEG_OC
        # A0 A1 B0 A2 B1 A3 B2 B3: A's first slots run while B's loads land
        order = [(0, 0), (0, 1), (1, 0), (0, 2), (1, 1), (0, 3), (1, 2), (1, 3)]
        slot_seg = [segs[si] for si, _ in order]
        slot_j = [j for _, j in order]
        slot_u = [None] * NSLOT
        slot_rcp = [None] * NSLOT
        slot_abf = [None] * NSLOT

        def mm1_softmax(s):
            seg = slot_seg[s]
            W, qs = seg["W"], seg["qs"]
            nQ = len(qs)
            lcol = slot_j[s] * P
            u_sb = upool.tile([P, W], F32, name=f"u_{s}", tag="u")
            nm = stats.tile([P, 1], F32, name=f"nm{s}", tag="nm")
            su = stats.tile([P, 4], F32, name=f"su{s}", tag="su")
            # Single softmax reference: the max of quarter 0 (>= 1024 Gaussian
            # scores, so later quarters cannot exceed it by anywhere near
            # exp's fp32 range; softmax is invariant to the reference).
            for q, (qoff, qtiles) in enumerate(qs):
                qw = sum(qtiles)
                s_ps = sps.tile([P, 1024], F32, name=f"s{s}_{q}", tag="s")
                for k in range(nKC):
                    toff = 0
                    for w in qtiles:
                        nc.tensor.matmul(
                            s_ps[:, toff:toff + w],
                            seg["oT_sb"][:, k, lcol:lcol + P],
                            seg["eT_q"][q][k][:, toff:toff + w],
                            start=(k == 0), stop=(k == nKC - 1))
                        toff += w
                if q == 0:
                    # reference = max over the first 512 scores (still >= 512
                    # Gaussian samples -> later values can't exceed it by
                    # anywhere near exp's fp32 range; softmax is shift-invariant)
                    nc.vector.reduce_max(nm, s_ps[:, :min(qw, 512)],
                                         axis=mybir.AxisListType.X, negate=True)
                nc.scalar.activation(out=u_sb[:, qoff:qoff + qw], in_=s_ps[:, :qw],
                                     func=mybir.ActivationFunctionType.Exp,
                                     bias=nm, scale=1.0,
                                     accum_out=su[:, q:q + 1])
            ssum = stats.tile([P, 1], F32, name=f"ssum{s}", tag="ssum")
            nc.vector.reduce_sum(ssum, su[:, :nQ], axis=mybir.AxisListType.X)
            corr = stats.tile([P, 1], F32, name=f"corr{s}", tag="corr")
            nc.vector.tensor_tensor(corr, u_sb[:, 0:1], seg["cnt_sb"],
                                    mybir.AluOpType.mult)
            sv = stats.tile([P, 1], F32, name=f"sv{s}", tag="sv")
            nc.vector.tensor_tensor(sv, ssum, corr, mybir.AluOpType.subtract)
            rcp = stats.tile([P, 1], F32, name=f"rcp{s}", tag="rcp")
            nc.vector.reciprocal(rcp, sv)
            slot_u[s] = u_sb
            slot_rcp[s] = rcp

        def finish(s):
            seg = slot_seg[s]
            W = seg["W"]
            lcol = slot_j[s] * P
            u_sb, rcp = slot_u[s], slot_rcp[s]
            nc.vector.tensor_scalar_mul(u_sb, u_sb, rcp)
            dq = nc.sync if s % 2 == 0 else nc.scalar
            dq.dma_start(out=seg["attn_d"][lcol:lcol + P, :], in_=u_sb)
            hs = min(((W // 2 + 511) // 512) * 512, W)
            if seg["name"] == "B" or hs == W:
                a_bf = bfpool.tile([P, W], BF16, name=f"abf{s}", tag="abf")
                nc.gpsimd.tensor_copy(out=a_bf, in_=u_sb)
                slot_abf[s] = [(0, W, a_bf)]
            else:
                a1 = bfpool.tile([P, hs], BF16, name=f"abf{s}a", tag="abfa")
                a2 = bfpool.tile([P, W - hs], BF16, name=f"abf{s}b", tag="abfb")
                nc.vector.tensor_copy(out=a1, in_=u_sb[:, :hs])
                nc.scalar.copy(out=a2, in_=u_sb[:, hs:])
                slot_abf[s] = [(0, hs, a1), (hs, W - hs, a2)]

        def transposes(s):
            seg = slot_seg[s]
            lcol = slot_j[s] * P
            gidx = 0
            for (hoff, hww, a_bf) in slot_abf[s]:
                ic0 = hoff // P
                nh_ic = hww // P
                for g in range(0, nh_ic, 4):
                    ng = min(4, nh_ic - g)
                    at_ps = atps.tile([P, 512], F32, name=f"at{s}_{ic0+g}", tag="at")
                    for j in range(ng):
                        nc.tensor.matmul(at_ps[:, j * P:(j + 1) * P],
                                         a_bf[:, (g + j) * P:(g + j + 1) * P],
                                         ident_bf)
                    dest = seg["atT"][slot_j[s]][:, ic0 + g:ic0 + g + ng, :]
                    srcv = at_ps[:, :ng * P].rearrange("p (a b) -> p a b", a=ng)
                    if gidx % 2 == 0:
                        nc.vector.tensor_copy(out=dest, in_=srcv)
                    else:
                        nc.scalar.copy(out=dest, in_=srcv)
                    gidx += 1

        def mm2_slot(s):
            seg = slot_seg[s]
            j = slot_j[s]
            nIC = seg["nIC"]
            atT = seg["atT"][j]
            ct_ps = ctps.tile([P, H], F32, name=f"ct{s}", tag="ct")
            for ic in range(nIC):
                nc.tensor.matmul(ct_ps, atT[:, ic, :], seg["e_sb"][:, ic, :],
                                 start=(ic == 0), stop=(ic == nIC - 1))
            c_sb = csb.tile([P, H], F32, name=f"csb{s}", tag="csb")
            if s % 2 == 0:
                nc.vector.tensor_copy(out=c_sb, in_=ct_ps)
            else:
                nc.scalar.copy(out=c_sb, in_=ct_ps)
            cq = nc.sync if s % 2 == 0 else nc.scalar
            cq.dma_start(out=seg["ctx_d"][j * P:(j + 1) * P, :], in_=c_sb)

        for t in range(NSLOT + SKEW):
            if t < NSLOT:
                mm1_softmax(t)
            if 0 <= t - 1 < NSLOT:
                finish(t - 1)
            if t >= SKEW:
                s = t - SKEW
                transposes(s)
                mm2_slot(s)

    nc.compile()
    return nc


def _ceil128(x):
    return max(256, ((x + P - 1) // P) * P)


def kernel(output: np.ndarray, encoder_outputs: np.ndarray,
           lengths: np.ndarray) -> tuple[np.ndarray, np.ndarray]:
    global _last_in_maps
    B, OUT, H = output.shape
    IN = encoder_outputs.shape[1]
    assert B == N_CORES
    lens = [max(1, min(int(l), IN)) for l in np.asarray(lengths)]

    order = sorted(range(B), key=lambda b: -lens[b])
    bigs, smalls = order[:4], order[7:3:-1]      # pair big[i] with small[i]
    W1 = _ceil128(max(lens[b] for b in bigs))
    W2 = _ceil128(max(lens[b] for b in smalls))

    key = (W1, W2, OUT, H)
    if key not in _program_cache:
        _program_cache[key] = _build_program(W1, W2, OUT, H)
    nc = _program_cache[key]

    def seg_inputs(b, W, nm):
        L = lens[b]
        E = np.asarray(encoder_outputs[b], dtype=np.float32)
        eT = np.empty((H, W), np.float16)
        eT[:, :L] = E[:L].T
        eT[:, L:] = E[0:1].T
        e_bf = np.zeros((W, H), ml_dtypes.bfloat16)
        e_bf[:L] = E[:L].astype(ml_dtypes.bfloat16)
        return {
            f"eT{nm}": eT, f"e{nm}": e_bf,
            f"cnt{nm}": np.full((P, 1), float(W - L), np.float32),
        }

    in_maps = []
    placement = []   # per core: (bigbatch, row slice, smallbatch, row slice)
    for p in range(4):
        bA, bB = bigs[p], smalls[p]
        iA = seg_inputs(bA, W1, "A")
        iB = seg_inputs(bB, W2, "B")
        OA = np.asarray(output[bA], np.float32).T.astype(np.float16)  # [H, 1024]
        OB = np.asarray(output[bB], np.float32).T.astype(np.float16)
        for half in range(2):
            sl = slice(half * 512, (half + 1) * 512)
            m = {"oTA": np.ascontiguousarray(OA[:, sl]),
                 "oTB": np.ascontiguousarray(OB[:, sl])}
            m.update(iA)
            m.update(iB)
            in_maps.append(m)
            placement.append((bA, sl, bB, sl))

    _last_in_maps = in_maps
    res = run_bass_kernel_spmd(nc, in_maps, list(range(N_CORES)))

    attn = np.zeros((B, OUT, IN), np.float32)
    context = np.empty((B, OUT, H), np.float32)
    for c, (bA, slA, bB, slB) in enumerate(placement):
        r = res.results[c]
        LA, LB = lens[bA], lens[bB]
        attn[bA, slA, :LA] = r["attnA"][:, :LA]
        attn[bB, slB, :LB] = r["attnB"][:, :LB]
        context[bA, slA] = r["ctxA"]
        context[bB, slB] = r["ctxB"]
    return (context, attn)
